# revision 10
# baseline (speedup 1.0000x reference)
"""MinLSTM Trainium2 kernel.

Problem: B=8, S=4096, In=512, H=512 (fp32).
    f_t = sigmoid(x @ W_f^T + b_f); i_t = sigmoid(x @ W_i^T + b_i)
    h_tilde = x @ W_h^T + b_h
    f_n = f_t / (f_t + i_t + eps); i_n = i_t / (f_t + i_t + eps)
    h_t = f_n * h_{t-1} + i_n * h_tilde   (scan over S)

Strategy: data-parallel over batch — 1 sample per NeuronCore (8 cores).
Per-core layout is transposed: [H on partitions, S on free dim] so that
  - gate matmuls run with W^T stationary (bf16) and x^T moving (bf16),
  - gate biases fuse into the sigmoid ACT op as per-partition bias,
  - the recurrence maps onto the native DVE tensor_tensor_scan
    (state = f_n * state + g along the free/time axis), chained across
    512-wide time chunks via initial=prev[:, -1:].

Host pre/post: transpose+cast x and W to bf16 blocked layouts, transpose
the [H, S] output back to [S, H].
"""

import numpy as np
import ml_dtypes

import concourse.bass as bass
import concourse.bacc as bacc
import concourse.tile as tile
from concourse import mybir
from concourse.bass import ts, ds
from concourse.bass_utils import run_bass_kernel_spmd

BF16 = ml_dtypes.bfloat16

B, S, IN, H = 8, 4096, 512, 512
KI = IN // 128        # 4 k-tiles of the contraction dim
HB = H // 128         # 4 h blocks (partition blocks)
TC = 512              # time-chunk (free dim per matmul / psum bank)
NT = S // TC          # 8 time chunks

_CACHE = {}


def build_minlstm_bass(repeat=1):
    nc = bacc.Bacc("TRN2", debug=False, num_devices=B)
    f32 = mybir.dt.float32
    bf16 = mybir.dt.bfloat16

    xT = nc.dram_tensor("xt", [KI, 128, S], bf16, kind="ExternalInput").ap()
    wfT = nc.dram_tensor("wft", [KI, 128, H], bf16, kind="ExternalInput").ap()
    wiT = nc.dram_tensor("wit", [KI, 128, H], bf16, kind="ExternalInput").ap()
    whT = nc.dram_tensor("wht", [KI, 128, H], bf16, kind="ExternalInput").ap()
    bfb = nc.dram_tensor("bfb", [128, HB], f32, kind="ExternalInput").ap()
    bib = nc.dram_tensor("bib", [128, HB], f32, kind="ExternalInput").ap()
    bhb = nc.dram_tensor("bhb", [128, HB], f32, kind="ExternalInput").ap()
    h0b = nc.dram_tensor("h0b", [128, HB], f32, kind="ExternalInput").ap()
    outT = nc.dram_tensor("outt", [HB, 128, S], f32, kind="ExternalOutput").ap()

    Sig = mybir.ActivationFunctionType.Sigmoid
    Ident = mybir.ActivationFunctionType.Identity
    Alu = mybir.AluOpType
    HALF = S // 2

    with tile.TileContext(nc) as tc, nc.allow_low_precision(reason="bf16 gates"):
        with (
            tc.tile_pool(name="const", bufs=1) as const,
            tc.tile_pool(name="ps", bufs=2, space="PSUM") as ps,
            tc.tile_pool(name="big", bufs=2) as big,
            tc.tile_pool(name="hout", bufs=2) as hout,
        ):
            wf_sb = const.tile([128, KI, H], bf16, tag="wf")
            wi_sb = const.tile([128, KI, H], bf16, tag="wi")
            wh_sb = const.tile([128, KI, H], bf16, tag="wh")
            x_sb = const.tile([128, KI, S], bf16, tag="x")
            for ki in range(KI):
                nc.sync.dma_start(out=wf_sb[:, ki, :], in_=wfT[ki, :, :])
                nc.sync.dma_start(out=wi_sb[:, ki, :], in_=wiT[ki, :, :])
                nc.sync.dma_start(out=wh_sb[:, ki, :], in_=whT[ki, :, :])
                nc.sync.dma_start(out=x_sb[:, ki, :], in_=xT[ki, :, :])
            bf_sb = const.tile([128, HB], f32, tag="bf")
            bi_sb = const.tile([128, HB], f32, tag="bi")
            bh_sb = const.tile([128, HB], f32, tag="bh")
            h0_sb = const.tile([128, HB], f32, tag="h0")
            nc.sync.dma_start(out=bf_sb, in_=bfb[:, :])
            nc.sync.dma_start(out=bi_sb, in_=bib[:, :])
            nc.sync.dma_start(out=bh_sb, in_=bhb[:, :])
            nc.sync.dma_start(out=h0_sb, in_=h0b[:, :])

            def body(_i=None):
                for hb in range(HB):
                    sf = big.tile([128, S], bf16, tag="sf")
                    si = big.tile([128, S], bf16, tag="si")
                    ht = big.tile([128, S], bf16, tag="ht")
                    for tci in range(NT):
                        pf = ps.tile([128, TC], f32, tag="pf")
                        pi = ps.tile([128, TC], f32, tag="pi")
                        ph = ps.tile([128, TC], f32, tag="ph")
                        for ki in range(KI):
                            st, sp = (ki == 0), (ki == KI - 1)
                            xk = x_sb[:, ki, ts(tci, TC)]
                            nc.tensor.matmul(
                                pf, wf_sb[:, ki, ds(hb * 128, 128)], xk,
                                start=st, stop=sp)
                            nc.tensor.matmul(
                                pi, wi_sb[:, ki, ds(hb * 128, 128)], xk,
                                start=st, stop=sp)
                            nc.tensor.matmul(
                                ph, wh_sb[:, ki, ds(hb * 128, 128)], xk,
                                start=st, stop=sp)
                        sl = ts(tci, TC)
                        nc.scalar.activation(
                            sf[:, sl], pf, Sig, bias=bf_sb[:, hb : hb + 1])
                        nc.scalar.activation(
                            si[:, sl], pi, Sig, bias=bi_sb[:, hb : hb + 1])
                        nc.scalar.activation(
                            ht[:, sl], ph, Ident, bias=bh_sb[:, hb : hb + 1])
                    fn = big.tile([128, S], bf16, tag="fn")
                    g = big.tile([128, S], bf16, tag="g")
                    rr = big.tile([128, S], bf16, tag="rr")
                    inf_ = big.tile([128, S], bf16, tag="inf")
                    for hf in range(2):
                        sl = ts(hf, HALF)
                        nc.vector.tensor_tensor(
                            rr[:, sl], sf[:, sl], si[:, sl], Alu.add)
                        nc.vector.reciprocal(rr[:, sl], rr[:, sl])
                        nc.vector.tensor_tensor(
                            fn[:, sl], sf[:, sl], rr[:, sl], Alu.mult)
                        nc.vector.tensor_scalar(
                            inf_[:, sl], fn[:, sl], 1.0, -1.0,
                            Alu.subtract, Alu.mult)
                        nc.vector.tensor_tensor(
                            g[:, sl], ht[:, sl], inf_[:, sl], Alu.mult)
                    hh = hout.tile([128, S], f32, tag="h")
                    nc.vector.tensor_tensor_scan(
                        hh, fn, g, h0_sb[:, hb : hb + 1], Alu.mult, Alu.add)
                    nc.sync.dma_start(out=outT[hb, :, :], in_=hh)

            if repeat == 1:
                body()
            else:
                with tc.For_i(0, repeat, 1) as _i:
                    body(_i)
    nc.compile()
    return nc


def _prep_core_inputs(x, h_0, W_f, b_f, W_i, b_i, W_h, b_h):
    """Build per-core input maps (host-side shard + layout transform)."""
    wft = np.ascontiguousarray(W_f.T.reshape(KI, 128, H).astype(BF16))
    wit = np.ascontiguousarray(W_i.T.reshape(KI, 128, H).astype(BF16))
    wht = np.ascontiguousarray(W_h.T.reshape(KI, 128, H).astype(BF16))
    bfb = np.ascontiguousarray(b_f.reshape(HB, 128).T.astype(np.float32))
    bib = np.ascontiguousarray(b_i.reshape(HB, 128).T.astype(np.float32))
    bhb = np.ascontiguousarray(b_h.reshape(HB, 128).T.astype(np.float32))
    in_maps = []
    for b in range(B):
        xt = np.ascontiguousarray(
            x[b].T.reshape(KI, 128, S).astype(BF16))
        h0b = np.ascontiguousarray(
            h_0[b].reshape(HB, 128).T.astype(np.float32))
        in_maps.append({
            "xt": xt, "wft": wft, "wit": wit, "wht": wht,
            "bfb": bfb, "bib": bib, "bhb": bhb, "h0b": h0b,
        })
    return in_maps


def _run(in_maps, trace=False):
    if "nc" not in _CACHE:
        _CACHE["nc"] = build_minlstm_bass()
    return run_bass_kernel_spmd(
        _CACHE["nc"], in_maps, core_ids=list(range(B)), trace=trace)


def kernel(x, h_0, W_f, b_f, W_i, b_i, W_h, b_h):
    x = np.asarray(x, dtype=np.float32)
    h_0 = np.asarray(h_0, dtype=np.float32)
    in_maps = _prep_core_inputs(
        x, h_0,
        np.asarray(W_f, np.float32), np.asarray(b_f, np.float32),
        np.asarray(W_i, np.float32), np.asarray(b_i, np.float32),
        np.asarray(W_h, np.float32), np.asarray(b_h, np.float32))
    res = _run(in_maps)
    out = np.empty((B, S, H), dtype=np.float32)
    for b in range(B):
        outt = res.results[b]["outt"]  # [HB, 128, S]
        out[b] = outt.reshape(H, S).T
    return out
